# revision 4
# baseline (speedup 1.0000x reference)
"""Binarized MLP (784 -> 1024 -> 1024 -> 1024 -> 10) on 8 TRN2 NeuronCores.

Data-parallel over the batch (16384 rows -> 2048 per core), weights replicated.

Math notes (these make the kernel both fast and numerically faithful):
  * Layers 1-2 outputs are only ever consumed through binarize(hardtanh(bn(h))).
    Since hardtanh preserves sign and bn here is (h - m) * rsqrt(v+eps) * g + be
    with g > 0, be == 0, the next-layer input is exactly sign(h + (b - m)).
    That is one ScalarE Sign activation with a per-partition bias, no bn needed.
  * fc2/fc3 multiply two +-1 operands -> exact in fp8(e4m3) with fp32 PSUM
    accumulation (integer partial sums, magnitude <= 1024).
  * fc1 keeps x at full precision via an exact fp16 hi/lo split:
    x = hi + lo with hi = fp16(x), lo = fp16(x - hi); products with +-1 weights
    are exact, so accuracy ~ fp32 matmul, at 2 bf16-rate passes.
  * fc4 + log_softmax: logits computed feature-major [10, B], PE-transposed to
    [B, 10]; log_softmax without max-subtraction (logits are small; exp is safe).
"""

import os
import numpy as np

N_CORES = 8
B_FULL = 16384
BS = B_FULL // N_CORES  # 2048 rows per core
IN_F = 784
K1C = 7                 # fc1 contraction chunks of 128 (784 padded to 896)
H = 1024
HC = 8                  # hidden chunks of 128
OUT_F = 10
NSPLIT = 4              # batch column chunks of 512
NB = BS // NSPLIT       # 512
BT = BS // 128          # 16 batch tiles of 128 for the output transpose

LAST_RESULT = None      # BassKernelResults of the most recent run (for test.py)

_PLAN = {}


def _build_nc():
    import concourse.bass as bass
    import concourse.mybir as mybir
    import concourse.tile as tile
    from concourse import bacc
    from concourse.bass import ts
    from concourse.masks import make_identity

    f32 = mybir.dt.float32
    f16 = mybir.dt.float16
    f8 = mybir.dt.float8e4
    AF = mybir.ActivationFunctionType
    ALU = mybir.AluOpType

    nc = bacc.Bacc(None)

    xhi_t = nc.dram_tensor("xhi", [K1C, 128, BS], f16, kind="ExternalInput")
    xlo_t = nc.dram_tensor("xlo", [K1C, 128, BS], f16, kind="ExternalInput")
    s1_t = nc.dram_tensor("s1t", [HC, K1C, 128, 128], f16, kind="ExternalInput")
    s2_t = nc.dram_tensor("s2t", [HC, HC, 128, 128], f8, kind="ExternalInput")
    s3_t = nc.dram_tensor("s3t", [HC, HC, 128, 128], f8, kind="ExternalInput")
    w4_t = nc.dram_tensor("w4t", [HC, 128, OUT_F], f16, kind="ExternalInput")
    b1_t = nc.dram_tensor("bias1", [H], f32, kind="ExternalInput")
    b2_t = nc.dram_tensor("bias2", [H], f32, kind="ExternalInput")
    sc3_t = nc.dram_tensor("sc3", [H], f32, kind="ExternalInput")
    sh3_t = nc.dram_tensor("sh3", [H], f32, kind="ExternalInput")
    b4_t = nc.dram_tensor("b4", [OUT_F], f32, kind="ExternalInput")
    y_t = nc.dram_tensor("y", [BS, OUT_F], f32, kind="ExternalOutput")

    with tile.TileContext(nc) as tc:
        with (
            tc.tile_pool(name="consts", bufs=1) as consts,
            tc.tile_pool(name="tmp", bufs=3) as tmp,
            tc.tile_pool(name="psmm", bufs=4, space="PSUM") as psmm,
            tc.tile_pool(name="ps4", bufs=2, space="PSUM") as ps4p,
            tc.tile_pool(name="pstr", bufs=2, space="PSUM") as pstrp,
        ):
            xhi_sb = consts.tile([128, K1C, BS], f16, tag="xhi")
            xlo_sb = consts.tile([128, K1C, BS], f16, tag="xlo")
            s1_sb = consts.tile([128, HC, K1C, 128], f16, tag="s1")
            s2_sb = consts.tile([128, HC, HC, 128], f8, tag="s2")
            s3_sb = consts.tile([128, HC, HC, 128], f8, tag="s3")
            w4_sb = consts.tile([128, HC, OUT_F], f16, tag="w4")
            b1v = consts.tile([128, HC], f32, tag="b1v")
            b2v = consts.tile([128, HC], f32, tag="b2v")
            sc3v = consts.tile([128, HC], f32, tag="sc3v")
            sh3v = consts.tile([128, HC], f32, tag="sh3v")
            b4bc = consts.tile([128, OUT_F], f32, tag="b4bc")
            ident = consts.tile([OUT_F, OUT_F], f32, tag="ident")
            act1 = consts.tile([128, HC, BS], f8, tag="act1")
            act2 = consts.tile([128, HC, BS], f8, tag="act2")
            act3 = consts.tile([128, HC, BS], f16, tag="act3")
            logits = consts.tile([OUT_F, BS], f32, tag="logits")
            lt = consts.tile([128, BT, OUT_F], f32, tag="lt")
            esb = consts.tile([128, BT, OUT_F], f32, tag="esb")
            lse = consts.tile([128, BT], f32, tag="lse")
            outf = consts.tile([128, BT, OUT_F], f32, tag="outf")

            # ---- input DMAs (x n=0 slices first so fc1 starts early) ----
            for n in range(NSPLIT):
                nsl = ts(n, NB)
                for k in range(K1C):
                    nc.sync.dma_start(out=xhi_sb[:, k, nsl], in_=xhi_t[k, :, nsl])
                    nc.sync.dma_start(out=xlo_sb[:, k, nsl], in_=xlo_t[k, :, nsl])
                if n == 0:
                    for m in range(HC):
                        nc.sync.dma_start(
                            out=s1_sb[:, m],
                            in_=s1_t[m].rearrange("k p c -> p k c"),
                        )
                    nc.sync.dma_start(
                        out=b1v, in_=b1_t[:].rearrange("(m p) -> p m", p=128)
                    )

            # ---- fc1: h1 = xT.T @ s1T  (feature-major out), sign -> act1 ----
            for n in range(NSPLIT):
                nsl = ts(n, NB)
                for m in range(HC):
                    ps = psmm.tile([128, NB], f32, tag="mm")
                    for k in range(K1C):
                        nc.tensor.matmul(
                            ps, s1_sb[:, m, k], xhi_sb[:, k, nsl],
                            start=(k == 0), stop=False,
                        )
                        nc.tensor.matmul(
                            ps, s1_sb[:, m, k], xlo_sb[:, k, nsl],
                            start=False, stop=(k == K1C - 1),
                        )
                    nc.scalar.activation(
                        act1[:, m, nsl], ps, AF.Sign, bias=b1v[:, m:m + 1]
                    )

            # weights for later layers (scheduler overlaps these with fc1)
            for m in range(HC):
                nc.sync.dma_start(
                    out=s2_sb[:, m], in_=s2_t[m].rearrange("k p c -> p k c")
                )
            nc.sync.dma_start(out=b2v, in_=b2_t[:].rearrange("(m p) -> p m", p=128))
            for m in range(HC):
                nc.sync.dma_start(
                    out=s3_sb[:, m], in_=s3_t[m].rearrange("k p c -> p k c")
                )
            nc.sync.dma_start(out=sc3v, in_=sc3_t[:].rearrange("(m p) -> p m", p=128))
            nc.sync.dma_start(out=sh3v, in_=sh3_t[:].rearrange("(m p) -> p m", p=128))
            nc.sync.dma_start(out=w4_sb, in_=w4_t.rearrange("k p o -> p k o"))
            b4_ap = b4_t[:]
            nc.sync.dma_start(
                out=b4bc,
                in_=bass.AP(tensor=b4_ap.tensor, offset=b4_ap.offset,
                            ap=[[0, 128]] + list(b4_ap.ap)),
            )
            make_identity(nc, ident)

            # ---- fc2: binary x binary (fp8), sign -> act2 ----
            for m in range(HC):
                for n in range(NSPLIT):
                    nsl = ts(n, NB)
                    ps = psmm.tile([128, NB], f32, tag="mm")
                    for k in range(HC):
                        nc.tensor.matmul(
                            ps, s2_sb[:, m, k], act1[:, k, nsl],
                            start=(k == 0), stop=(k == HC - 1),
                        )
                    nc.scalar.activation(
                        act2[:, m, nsl], ps, AF.Sign, bias=b2v[:, m:m + 1]
                    )

            # ---- fc3: binary x binary (fp8), bn affine + hardtanh -> act3 ----
            for m in range(HC):
                for n in range(NSPLIT):
                    nsl = ts(n, NB)
                    ps = psmm.tile([128, NB], f32, tag="mm")
                    for k in range(HC):
                        nc.tensor.matmul(
                            ps, s3_sb[:, m, k], act2[:, k, nsl],
                            start=(k == 0), stop=(k == HC - 1),
                        )
                    t = tmp.tile([128, NB], f32, tag="t3")
                    nc.scalar.activation(
                        t, ps, AF.Identity,
                        bias=sh3v[:, m:m + 1], scale=sc3v[:, m:m + 1],
                    )
                    nc.vector.tensor_scalar(
                        out=act3[:, m, nsl], in0=t,
                        scalar1=-1.0, scalar2=1.0,
                        op0=ALU.max, op1=ALU.min,
                    )

            # ---- fc4: logits[10, BS] ----
            for n in range(NSPLIT):
                nsl = ts(n, NB)
                ps = ps4p.tile([OUT_F, NB], f32, tag="mm4")
                for k in range(HC):
                    nc.tensor.matmul(
                        ps, w4_sb[:, k], act3[:, k, nsl],
                        start=(k == 0), stop=(k == HC - 1),
                    )
                nc.scalar.copy(logits[:, nsl], ps)

            # ---- transpose to [B, 10] and log_softmax ----
            for i in range(BT):
                pt = pstrp.tile([128, OUT_F], f32, tag="tr")
                nc.tensor.transpose(pt, logits[:, ts(i, 128)], ident)
                nc.vector.tensor_copy(lt[:, i], pt)
            b4r = b4bc[:]
            nc.vector.tensor_tensor(
                out=lt, in0=lt,
                in1=bass.AP(tensor=b4r.tensor, offset=b4r.offset,
                            ap=[b4r.ap[0], [0, BT], b4r.ap[1]]),
                op=ALU.add,
            )
            nc.scalar.activation(esb, lt, AF.Exp)
            nc.vector.tensor_reduce(
                out=lse, in_=esb, axis=mybir.AxisListType.X, op=ALU.add
            )
            nc.scalar.activation(lse, lse, AF.Ln)
            for g in range(BT):
                nc.vector.tensor_scalar_sub(outf[:, g], lt[:, g], lse[:, g:g + 1])
            nc.sync.dma_start(
                out=y_t.rearrange("(i p) o -> p i o", p=128), in_=outf
            )

    nc.finalize()
    return nc


def _host_prep(inputs):
    """Shard x, binarize/lay out weights, fold bn into sign biases."""
    import ml_dtypes

    f16 = np.float16
    f8 = ml_dtypes.float8_e4m3

    x = np.asarray(inputs["x"], np.float32)
    w1 = np.asarray(inputs["w1"], np.float32)
    w2 = np.asarray(inputs["w2"], np.float32)
    w3 = np.asarray(inputs["w3"], np.float32)
    w4 = np.asarray(inputs["w4"], np.float32)
    b1 = np.asarray(inputs["b1"], np.float32)
    b2 = np.asarray(inputs["b2"], np.float32)
    b3 = np.asarray(inputs["b3"], np.float32)
    b4 = np.asarray(inputs["b4"], np.float32)

    EPS = np.float64(1e-5)

    def gv(i):
        return (np.asarray(inputs[f"g{i}"], np.float32),
                np.asarray(inputs[f"be{i}"], np.float32),
                np.asarray(inputs[f"m{i}"], np.float32),
                np.asarray(inputs[f"v{i}"], np.float32))

    g1, be1, m1, v1 = gv(1)
    g2, be2, m2, v2 = gv(2)
    g3, be3, m3, v3 = gv(3)
    # sign(bn(h)) == sign(h + (b - m)) requires gamma > 0 and beta == 0
    assert np.all(g1 > 0) and np.all(be1 == 0), "unsupported bn1 params"
    assert np.all(g2 > 0) and np.all(be2 == 0), "unsupported bn2 params"

    bias1 = (b1 - m1).astype(np.float32)
    bias2 = (b2 - m2).astype(np.float32)
    r3 = 1.0 / np.sqrt(v3.astype(np.float64) + EPS)
    sc3 = (r3 * g3).astype(np.float32)
    sh3 = ((b3 - m3).astype(np.float64) * r3 * g3 + be3).astype(np.float32)

    def wlay(w, kc, dt):  # [out, in] -> [m, k, 128p(in), 128c(out)]
        st = np.sign(w).T.astype(np.float32)            # [in, out]
        kin = kc * 128
        if st.shape[0] < kin:
            st = np.pad(st, ((0, kin - st.shape[0]), (0, 0)))
        mo = st.shape[1] // 128
        return np.ascontiguousarray(
            st.reshape(kc, 128, mo, 128).transpose(2, 0, 1, 3)
        ).astype(dt)

    s1t = wlay(w1, K1C, f16)
    s2t = wlay(w2, HC, f8)
    s3t = wlay(w3, HC, f8)
    w4t = np.ascontiguousarray(w4.T.astype(f16)).reshape(HC, 128, OUT_F)

    shared = dict(s1t=s1t, s2t=s2t, s3t=s3t, w4t=w4t,
                  bias1=bias1, bias2=bias2, sc3=sc3, sh3=sh3, b4=b4)
    in_maps = []
    for c in range(N_CORES):
        xs = x[c * BS:(c + 1) * BS]                     # [2048, 784]
        xt = np.zeros((K1C * 128, BS), np.float32)
        xt[:IN_F] = xs.T
        xhi = xt.astype(f16)
        xlo = (xt - xhi.astype(np.float32)).astype(f16)
        m = dict(shared)
        m["xhi"] = np.ascontiguousarray(xhi).reshape(K1C, 128, BS)
        m["xlo"] = np.ascontiguousarray(xlo).reshape(K1C, 128, BS)
        in_maps.append(m)
    return in_maps


def kernel(**inputs):
    global LAST_RESULT
    from concourse.bass_utils import run_bass_kernel_spmd

    if "nc" not in _PLAN:
        _PLAN["nc"] = _build_nc()
    nc = _PLAN["nc"]

    in_maps = _host_prep(inputs)
    br = run_bass_kernel_spmd(
        nc, in_maps, list(range(N_CORES)),
        tmpdir=os.environ.get("KERNEL_TMPDIR") or None,
    )
    LAST_RESULT = br
    out = np.concatenate([br.results[c]["y"] for c in range(N_CORES)], axis=0)
    return out.astype(np.float32)


# revision 6
# speedup vs baseline: 1.4599x; 1.4599x over previous
"""Binarized MLP (784 -> 1024 -> 1024 -> 1024 -> 10) on 8 TRN2 NeuronCores.

Data-parallel over the batch (16384 rows -> 2048 per core), weights replicated.

Math notes (these make the kernel both fast and numerically faithful):
  * Layers 1-2 outputs are only ever consumed through binarize(hardtanh(bn(h))).
    Since hardtanh preserves sign and bn here is (h - m) * rsqrt(v+eps) * g + be
    with g > 0, be == 0, the next-layer input is exactly sign(h + (b - m)).
    That is one ScalarE Sign activation with a per-partition bias, no bn needed.
  * fc2/fc3 multiply two +-1 operands -> exact in fp8(e4m3) with fp32 PSUM
    accumulation (integer partial sums, magnitude <= 1024). DoubleRow perf mode
    contracts two 128-row chunks per pass (2 fp8 weights per PE cell).
  * fc1 keeps x at full precision via an exact fp16 hi/lo split:
    x = hi + lo with hi = fp16(x), lo = fp16(x - hi); products with +-1 weights
    are exact, so accuracy ~ fp32 matmul, at 2 bf16-rate passes.
  * fc4 + log_softmax: logits computed feature-major [10, B], PE-transposed to
    [B, 10]; log_softmax without max-subtraction (logits are small; exp is safe).

Loop order: weights stationary per (m, k); all 4 batch column chunks stream
per weight load (amortizes LDWEIGHTS). 4 PSUM banks accumulate per m-tile,
8-slot pool double-buffers across m-tiles.
"""

import os
import numpy as np

N_CORES = 8
B_FULL = 16384
BS = B_FULL // N_CORES  # 2048 rows per core
IN_F = 784
K1C = 7                 # fc1 contraction chunks of 128 (784 padded to 896)
H = 1024
HC = 8                  # hidden chunks of 128
OUT_F = 10
NSPLIT = 4              # batch column chunks of 512
NB = BS // NSPLIT       # 512
BT = BS // 128          # 16 batch tiles of 128 for the output transpose

LAST_RESULT = None      # BassKernelResults of the most recent run (for test.py)

_PLAN = {}


def _build_nc():
    import concourse.bass as bass
    import concourse.mybir as mybir
    import concourse.tile as tile
    from concourse import bacc
    from concourse.bass import ts
    from concourse.masks import make_identity

    f32 = mybir.dt.float32
    f16 = mybir.dt.float16
    f8 = mybir.dt.float8e4
    AF = mybir.ActivationFunctionType
    ALU = mybir.AluOpType
    DR = mybir.MatmulPerfMode.DoubleRow

    nc = bacc.Bacc(None)

    xhi_t = nc.dram_tensor("xhi", [K1C, 128, BS], f16, kind="ExternalInput")
    xlo_t = nc.dram_tensor("xlo", [K1C, 128, BS], f16, kind="ExternalInput")
    s1_t = nc.dram_tensor("s1t", [HC, K1C, 128, 128], f16, kind="ExternalInput")
    s2_t = nc.dram_tensor("s2t", [HC, HC, 128, 128], f8, kind="ExternalInput")
    s3_t = nc.dram_tensor("s3t", [HC, HC, 128, 128], f8, kind="ExternalInput")
    w4_t = nc.dram_tensor("w4t", [HC, 128, OUT_F], f16, kind="ExternalInput")
    b1_t = nc.dram_tensor("bias1", [H], f32, kind="ExternalInput")
    b2_t = nc.dram_tensor("bias2", [H], f32, kind="ExternalInput")
    sc3_t = nc.dram_tensor("sc3", [H], f32, kind="ExternalInput")
    sh3_t = nc.dram_tensor("sh3", [H], f32, kind="ExternalInput")
    b4_t = nc.dram_tensor("b4", [OUT_F], f32, kind="ExternalInput")
    y_t = nc.dram_tensor("y", [BS, OUT_F], f32, kind="ExternalOutput")

    with tile.TileContext(nc) as tc:
        with (
            tc.tile_pool(name="consts", bufs=1) as consts,
            tc.tile_pool(name="tmp", bufs=4) as tmp,
            tc.tile_pool(name="psum", bufs=8, space="PSUM") as psum,
        ):
            xhi_sb = consts.tile([128, K1C, BS], f16, tag="xhi")
            xlo_sb = consts.tile([128, K1C, BS], f16, tag="xlo")
            s1_sb = consts.tile([128, HC, K1C, 128], f16, tag="s1")
            s2_sb = consts.tile([128, HC, HC, 128], f8, tag="s2")
            s3_sb = consts.tile([128, HC, HC, 128], f8, tag="s3")
            w4_sb = consts.tile([128, HC, OUT_F], f16, tag="w4")
            b1v = consts.tile([128, HC], f32, tag="b1v")
            b2v = consts.tile([128, HC], f32, tag="b2v")
            sc3v = consts.tile([128, HC], f32, tag="sc3v")
            sh3v = consts.tile([128, HC], f32, tag="sh3v")
            b4bc = consts.tile([128, OUT_F], f32, tag="b4bc")
            ident = consts.tile([OUT_F, OUT_F], f32, tag="ident")
            act1 = consts.tile([128, HC, BS], f8, tag="act1")
            act2 = consts.tile([128, HC, BS], f8, tag="act2")
            act3 = consts.tile([128, HC, BS], f16, tag="act3")
            logits = consts.tile([OUT_F, BS], f32, tag="logits")
            lt = consts.tile([128, BT, OUT_F], f32, tag="lt")
            esb = consts.tile([128, BT, OUT_F], f32, tag="esb")
            lse = consts.tile([128, BT], f32, tag="lse")
            outf = consts.tile([128, BT, OUT_F], f32, tag="outf")

            # ---- input DMAs: first weights for m=0, then x per k-chunk ----
            # hi/lo go to different issuing engines so enqueue parallelizes.
            nc.sync.dma_start(out=s1_sb[:, 0], in_=s1_t[0].rearrange("k p c -> p k c"))
            nc.gpsimd.dma_start(out=b1v, in_=b1_t[:].rearrange("(m p) -> p m", p=128))
            for k in range(K1C):
                nc.sync.dma_start(out=xhi_sb[:, k], in_=xhi_t[k])
                nc.gpsimd.dma_start(out=xlo_sb[:, k], in_=xlo_t[k])
            for m in range(1, HC):
                nc.sync.dma_start(
                    out=s1_sb[:, m], in_=s1_t[m].rearrange("k p c -> p k c")
                )

            # ---- fc1: h1 = xT.T @ s1T (feature-major), sign -> act1 ----
            for m in range(HC):
                pss = [psum.tile([128, NB], f32, tag="mm", name="ps") for _ in range(NSPLIT)]
                for k in range(K1C):
                    for n in range(NSPLIT):
                        nsl = ts(n, NB)
                        nc.tensor.matmul(
                            pss[n], s1_sb[:, m, k], xhi_sb[:, k, nsl],
                            start=(k == 0), stop=False,
                        )
                        nc.tensor.matmul(
                            pss[n], s1_sb[:, m, k], xlo_sb[:, k, nsl],
                            start=False, stop=(k == K1C - 1),
                        )
                for n in range(NSPLIT):
                    nc.scalar.activation(
                        act1[:, m, ts(n, NB)], pss[n], AF.Sign, bias=b1v[:, m:m + 1]
                    )

            # later-layer weights (scheduler overlaps these DMAs with fc1)
            for m in range(HC):
                nc.gpsimd.dma_start(
                    out=s2_sb[:, m], in_=s2_t[m].rearrange("k p c -> p k c")
                )
            nc.sync.dma_start(out=b2v, in_=b2_t[:].rearrange("(m p) -> p m", p=128))
            for m in range(HC):
                nc.gpsimd.dma_start(
                    out=s3_sb[:, m], in_=s3_t[m].rearrange("k p c -> p k c")
                )
            nc.sync.dma_start(out=sc3v, in_=sc3_t[:].rearrange("(m p) -> p m", p=128))
            nc.sync.dma_start(out=sh3v, in_=sh3_t[:].rearrange("(m p) -> p m", p=128))
            nc.sync.dma_start(out=w4_sb, in_=w4_t.rearrange("k p o -> p k o"))
            b4_ap = b4_t[:]
            nc.sync.dma_start(
                out=b4bc,
                in_=bass.AP(tensor=b4_ap.tensor, offset=b4_ap.offset,
                            ap=[[0, 128]] + list(b4_ap.ap)),
            )
            make_identity(nc, ident)

            # ---- fc2: binary x binary, fp8 DoubleRow, sign -> act2 ----
            for m in range(HC):
                pss = [psum.tile([128, NB], f32, tag="mm", name="ps") for _ in range(NSPLIT)]
                for kk in range(HC // 2):
                    ksl = slice(2 * kk, 2 * kk + 2)
                    for n in range(NSPLIT):
                        nc.tensor.matmul(
                            pss[n], s2_sb[:, m, ksl], act1[:, ksl, ts(n, NB)],
                            start=(kk == 0), stop=(kk == HC // 2 - 1),
                            perf_mode=DR,
                        )
                for n in range(NSPLIT):
                    nc.scalar.activation(
                        act2[:, m, ts(n, NB)], pss[n], AF.Sign, bias=b2v[:, m:m + 1]
                    )

            # ---- fc3: fp8 DoubleRow, bn affine + hardtanh -> act3 (DVE) ----
            for m in range(HC):
                pss = [psum.tile([128, NB], f32, tag="mm", name="ps") for _ in range(NSPLIT)]
                for kk in range(HC // 2):
                    ksl = slice(2 * kk, 2 * kk + 2)
                    for n in range(NSPLIT):
                        nc.tensor.matmul(
                            pss[n], s3_sb[:, m, ksl], act2[:, ksl, ts(n, NB)],
                            start=(kk == 0), stop=(kk == HC // 2 - 1),
                            perf_mode=DR,
                        )
                for n in range(NSPLIT):
                    t = tmp.tile([128, NB], f32, tag="t3")
                    nc.vector.tensor_scalar(
                        out=t, in0=pss[n],
                        scalar1=sc3v[:, m:m + 1], scalar2=sh3v[:, m:m + 1],
                        op0=ALU.mult, op1=ALU.add,
                    )
                    nc.vector.tensor_scalar(
                        out=act3[:, m, ts(n, NB)], in0=t,
                        scalar1=-1.0, scalar2=1.0,
                        op0=ALU.max, op1=ALU.min,
                    )

            # ---- fc4: logits[10, BS] ----
            ps4 = [psum.tile([OUT_F, NB], f32, tag="mm", name="ps4") for _ in range(NSPLIT)]
            for k in range(HC):
                for n in range(NSPLIT):
                    nc.tensor.matmul(
                        ps4[n], w4_sb[:, k], act3[:, k, ts(n, NB)],
                        start=(k == 0), stop=(k == HC - 1),
                    )
            for n in range(NSPLIT):
                nc.scalar.copy(logits[:, ts(n, NB)], ps4[n])

            # ---- transpose to [B, 10] and log_softmax ----
            for i in range(BT):
                pt = psum.tile([128, OUT_F], f32, tag="mm")
                nc.tensor.transpose(pt, logits[:, ts(i, 128)], ident)
                nc.vector.tensor_copy(lt[:, i], pt)
            b4r = b4bc[:]
            nc.vector.tensor_tensor(
                out=lt, in0=lt,
                in1=bass.AP(tensor=b4r.tensor, offset=b4r.offset,
                            ap=[b4r.ap[0], [0, BT], b4r.ap[1]]),
                op=ALU.add,
            )
            nc.scalar.activation(esb, lt, AF.Exp)
            nc.vector.tensor_reduce(
                out=lse, in_=esb, axis=mybir.AxisListType.X, op=ALU.add
            )
            nc.scalar.activation(lse, lse, AF.Ln)
            for g in range(BT):
                nc.vector.tensor_scalar_sub(outf[:, g], lt[:, g], lse[:, g:g + 1])
            nc.sync.dma_start(
                out=y_t.rearrange("(i p) o -> p i o", p=128), in_=outf
            )

    nc.finalize()
    return nc


def _host_prep(inputs):
    """Shard x, binarize/lay out weights, fold bn into sign biases."""
    import ml_dtypes

    f16 = np.float16
    f8 = ml_dtypes.float8_e4m3

    x = np.asarray(inputs["x"], np.float32)
    w1 = np.asarray(inputs["w1"], np.float32)
    w2 = np.asarray(inputs["w2"], np.float32)
    w3 = np.asarray(inputs["w3"], np.float32)
    w4 = np.asarray(inputs["w4"], np.float32)
    b1 = np.asarray(inputs["b1"], np.float32)
    b2 = np.asarray(inputs["b2"], np.float32)
    b3 = np.asarray(inputs["b3"], np.float32)
    b4 = np.asarray(inputs["b4"], np.float32)

    EPS = np.float64(1e-5)

    def gv(i):
        return (np.asarray(inputs[f"g{i}"], np.float32),
                np.asarray(inputs[f"be{i}"], np.float32),
                np.asarray(inputs[f"m{i}"], np.float32),
                np.asarray(inputs[f"v{i}"], np.float32))

    g1, be1, m1, v1 = gv(1)
    g2, be2, m2, v2 = gv(2)
    g3, be3, m3, v3 = gv(3)
    # sign(bn(h)) == sign(h + (b - m)) requires gamma > 0 and beta == 0
    assert np.all(g1 > 0) and np.all(be1 == 0), "unsupported bn1 params"
    assert np.all(g2 > 0) and np.all(be2 == 0), "unsupported bn2 params"

    bias1 = (b1 - m1).astype(np.float32)
    bias2 = (b2 - m2).astype(np.float32)
    r3 = 1.0 / np.sqrt(v3.astype(np.float64) + EPS)
    sc3 = (r3 * g3).astype(np.float32)
    sh3 = ((b3 - m3).astype(np.float64) * r3 * g3 + be3).astype(np.float32)

    def wlay(w, kc, dt):  # [out, in] -> [m, k, 128p(in), 128c(out)]
        st = np.sign(w).T.astype(np.float32)            # [in, out]
        kin = kc * 128
        if st.shape[0] < kin:
            st = np.pad(st, ((0, kin - st.shape[0]), (0, 0)))
        mo = st.shape[1] // 128
        return np.ascontiguousarray(
            st.reshape(kc, 128, mo, 128).transpose(2, 0, 1, 3)
        ).astype(dt)

    s1t = wlay(w1, K1C, f16)
    s2t = wlay(w2, HC, f8)
    s3t = wlay(w3, HC, f8)
    w4t = np.ascontiguousarray(w4.T.astype(f16)).reshape(HC, 128, OUT_F)

    shared = dict(s1t=s1t, s2t=s2t, s3t=s3t, w4t=w4t,
                  bias1=bias1, bias2=bias2, sc3=sc3, sh3=sh3, b4=b4)
    in_maps = []
    for c in range(N_CORES):
        xs = x[c * BS:(c + 1) * BS]                     # [2048, 784]
        xt = np.zeros((K1C * 128, BS), np.float32)
        xt[:IN_F] = xs.T
        xhi = xt.astype(f16)
        xlo = (xt - xhi.astype(np.float32)).astype(f16)
        m = dict(shared)
        m["xhi"] = np.ascontiguousarray(xhi).reshape(K1C, 128, BS)
        m["xlo"] = np.ascontiguousarray(xlo).reshape(K1C, 128, BS)
        in_maps.append(m)
    return in_maps


def kernel(**inputs):
    global LAST_RESULT
    from concourse.bass_utils import run_bass_kernel_spmd

    if "nc" not in _PLAN:
        _PLAN["nc"] = _build_nc()
    nc = _PLAN["nc"]

    in_maps = _host_prep(inputs)
    br = run_bass_kernel_spmd(
        nc, in_maps, list(range(N_CORES)),
        tmpdir=os.environ.get("KERNEL_TMPDIR") or None,
    )
    LAST_RESULT = br
    out = np.concatenate([br.results[c]["y"] for c in range(N_CORES)], axis=0)
    return out.astype(np.float32)
